# revision 4
# baseline (speedup 1.0000x reference)
"""AttentionProbe (B=32, N=4096, D=1024) on 8 Trainium2 NeuronCores.

Sharding: data-parallel over batch B; each core owns 4 batches and streams
its x slice [4, 4096, 1024] f32 (64 MB) through SBUF exactly once.

Per [128, 1024] tile of x:
  - DVE tensor_tensor_reduce: logits col = sum_d(x * q_bcast) / sqrt(D)
  - ACT exp (no max-subtraction needed: logits ~ N(0, 1/1024))
  - 8 PE matmuls: pooledT[:, j] += x[:, 128j:128j+128].T @ exp(logits) col
Per batch tail: denom via PE ones-matmul, reciprocal on DVE, broadcast via
PE, alpha = exp(logits).T * (1/denom) transposed on PE and DMA'd out,
scores = pooledT . head_wT + head_b via a small TTR + PE partition-sum.
"""

import numpy as np

import concourse.mybir as mybir
import concourse.tile as tile
from concourse import bacc
from concourse.bass_utils import run_bass_kernel_spmd
from concourse.masks import make_identity

B, N, D = 32, 4096, 1024
NCORES = 8
BL = B // NCORES   # batches per core
P = 128
T = N // P         # n-tiles per batch
DJ = D // P        # d-chunks
INV_SQRT_D = 1.0 / 32.0

f32 = mybir.dt.float32

_CACHE = {}
LAST_RESULTS = None


def build_kernel(BL=BL, N=N, D=D, num_devices=NCORES):
    T = N // P
    DJ = D // P
    INV_SQRT_D = 1.0 / float(np.sqrt(D))
    nc = bacc.Bacc(
        "TRN2",
        target_bir_lowering=False,
        debug=False,
        enable_asserts=False,
        num_devices=num_devices,
    )
    x_d = nc.dram_tensor("x", [BL, N, D], f32, kind="ExternalInput").ap()
    qb_d = nc.dram_tensor("qb", [P, D], f32, kind="ExternalInput").ap()
    hwt_d = nc.dram_tensor("hwt", [P, DJ], f32, kind="ExternalInput").ap()
    hb_d = nc.dram_tensor("hb", [1, 1], f32, kind="ExternalInput").ap()
    alpha_d = nc.dram_tensor("alpha", [BL, T, P], f32, kind="ExternalOutput").ap()
    scores_d = nc.dram_tensor("scores", [1, BL], f32, kind="ExternalOutput").ap()

    Alu = mybir.AluOpType
    Act = mybir.ActivationFunctionType

    with tile.TileContext(nc) as tc:
        with (
            tc.tile_pool(name="consts", bufs=1) as consts,
            tc.tile_pool(name="xp", bufs=16) as xp,
            tc.tile_pool(name="lw", bufs=2) as lw,
            tc.tile_pool(name="small", bufs=2) as small,
            tc.tile_pool(name="outp", bufs=2) as outp,
            tc.tile_pool(name="pspool", bufs=2, space="PSUM") as psp,
            tc.tile_pool(name="pssmall", bufs=1, space="PSUM") as pss,
        ):
            qb = consts.tile([P, D], f32)
            nc.scalar.dma_start(qb, qb_d)
            hwt = consts.tile([P, DJ], f32)
            nc.scalar.dma_start(hwt, hwt_d)
            hb = consts.tile([1, 1], f32)
            nc.scalar.dma_start(hb, hb_d)
            ident = consts.tile([P, P], f32)
            make_identity(nc, ident)
            ones_col = consts.tile([P, 1], f32)
            nc.vector.memset(ones_col, 1.0)
            ones_row = consts.tile([1, P], f32)
            nc.vector.memset(ones_row, 1.0)
            dummy = consts.tile([P, 1], f32)        # throwaway TTR `out`
            scores_sb = consts.tile([1, BL], f32)   # persistent

            for b in range(BL):
                L = lw.tile([P, T], f32, tag="L")
                W = lw.tile([P, T], f32, tag="W")
                pooledT_ps = psp.tile([P, DJ], f32, tag="pooledT")
                for t in range(T):
                    xs = xp.tile([P, D], f32, tag="x")
                    nc.sync.dma_start(xs, x_d[b, t * P:(t + 1) * P, :])
                    nc.vector.scalar_tensor_tensor(
                        out=dummy.broadcast_to((P, D)),
                        in0=xs,
                        scalar=INV_SQRT_D,
                        in1=qb,
                        op0=Alu.mult,
                        op1=Alu.mult,
                        accum_out=L[:, t:t + 1],
                    )
                    nc.scalar.activation(W[:, t:t + 1], L[:, t:t + 1], Act.Exp)
                    for j in range(DJ):
                        nc.tensor.matmul(
                            pooledT_ps[:, j:j + 1],
                            lhsT=xs[:, j * P:(j + 1) * P],
                            rhs=W[:, t:t + 1],
                            start=(t == 0 and j == 0),
                            stop=(t == T - 1 and j == DJ - 1),
                        )

                # ---- per-batch tail ----
                s_col = small.tile([P, 1], f32, tag="s_col")
                nc.vector.tensor_reduce(s_col, W, mybir.AxisListType.X, Alu.add)
                denom_ps = pss.tile([1, 1], f32, tag="denom")
                nc.tensor.matmul(denom_ps, lhsT=ones_col, rhs=s_col, start=True, stop=True)
                denom_sb = small.tile([1, 1], f32, tag="denom_sb")
                nc.scalar.copy(denom_sb, denom_ps)
                recip_sb = small.tile([1, 1], f32, tag="recip_sb")
                nc.vector.reciprocal(recip_sb, denom_sb)
                bcast_ps = pss.tile([P, 1], f32, tag="bcast")
                nc.tensor.matmul(bcast_ps, lhsT=ones_row, rhs=recip_sb, start=True, stop=True)
                recip_col = small.tile([P, 1], f32, tag="recip_col")
                nc.scalar.copy(recip_col, bcast_ps)

                alphaT_ps = psp.tile([T, P], f32, tag="alphaT")
                nc.tensor.transpose(alphaT_ps, W, ident)
                alphaT_sb = outp.tile([T, P], f32, tag="alphaT_sb")
                nc.scalar.activation(
                    alphaT_sb, alphaT_ps, Act.Copy, scale=recip_col[0:T, :]
                )
                nc.scalar.dma_start(alpha_d[b], alphaT_sb)

                pooledT_sb = small.tile([P, DJ], f32, tag="pooledT_sb")
                nc.scalar.activation(pooledT_sb, pooledT_ps, Act.Copy, scale=recip_col)
                ps_col = small.tile([P, 1], f32, tag="ps_col")
                nc.vector.scalar_tensor_tensor(
                    out=dummy.broadcast_to((P, DJ)),
                    in0=pooledT_sb,
                    scalar=1.0,
                    in1=hwt,
                    op0=Alu.mult,
                    op1=Alu.mult,
                    accum_out=ps_col,
                )
                score_ps = pss.tile([1, 1], f32, tag="score")
                nc.tensor.matmul(score_ps, lhsT=ones_col, rhs=ps_col, start=True, stop=True)
                nc.scalar.activation(
                    scores_sb[:, b:b + 1], score_ps, Act.Identity, bias=hb, scale=1.0
                )

            nc.scalar.dma_start(scores_d, scores_sb)

    nc.compile()
    return nc


def kernel(x_final, x_full, mask, q, head_w, head_b, trace=False):
    global LAST_RESULTS
    if "nc" not in _CACHE:
        _CACHE["nc"] = build_kernel()
    nc = _CACHE["nc"]

    x_full = np.asarray(x_full, dtype=np.float32)
    q = np.asarray(q, dtype=np.float32).reshape(D)
    head_w = np.asarray(head_w, dtype=np.float32).reshape(D)
    head_b = np.asarray(head_b, dtype=np.float32).reshape(1, 1)

    qb = np.ascontiguousarray(np.broadcast_to(q, (P, D)))
    hwt = np.ascontiguousarray(head_w.reshape(DJ, P).T)

    in_maps = []
    for c in range(NCORES):
        in_maps.append(
            {
                "x": x_full[c * BL:(c + 1) * BL],
                "qb": qb,
                "hwt": hwt,
                "hb": head_b,
            }
        )

    res = run_bass_kernel_spmd(
        nc, in_maps, core_ids=list(range(NCORES)), trace=trace
    )
    LAST_RESULTS = res

    scores = np.concatenate(
        [res.results[c]["scores"].reshape(BL) for c in range(NCORES)]
    )
    alpha = np.concatenate(
        [res.results[c]["alpha"].reshape(BL, N) for c in range(NCORES)], axis=0
    )
    return scores, alpha


# revision 17
# speedup vs baseline: 1.0118x; 1.0118x over previous
"""AttentionProbe (B=32, N=4096, D=1024) on 8 Trainium2 NeuronCores.

Sharding: data-parallel over batch B; each core owns 4 batches and streams
its x slice [4, 4096, 1024] f32 (64 MB) through SBUF exactly once.

Per [128, 1024] tile of x:
  - DVE tensor_tensor_reduce: logits col = sum_d(x * q_bcast) / sqrt(D)
  - ACT exp (no max-subtraction needed: logits ~ N(0, 1/1024))
  - 8 PE matmuls: pooledT[:, j] += x[:, 128j:128j+128].T @ exp(logits) col
Per batch tail: denom via PE ones-matmul, reciprocal on DVE, broadcast via
PE, alpha = exp(logits).T * (1/denom) transposed on PE and DMA'd out,
scores = pooledT . head_wT + head_b via a small TTR + PE partition-sum.
"""

import numpy as np

import concourse.mybir as mybir
import concourse.tile as tile
from concourse import bacc
from concourse.bass_utils import run_bass_kernel_spmd
from concourse.masks import make_identity

B, N, D = 32, 4096, 1024
NCORES = 8
BL = B // NCORES   # batches per core
P = 128
T = N // P         # n-tiles per batch
DJ = D // P        # d-chunks
INV_SQRT_D = 1.0 / 32.0

f32 = mybir.dt.float32

_CACHE = {}
LAST_RESULTS = None


def build_kernel(BL=BL, N=N, D=D, num_devices=NCORES, reps=1, variant="full"):
    """variant: 'full' | 'nope' (skip PE work) | 'nodve' (skip STT/exp) |
    'dmaonly' (only the x stream)."""
    T = N // P
    DJ = D // P
    INV_SQRT_D = 1.0 / float(np.sqrt(D))
    do_dve = variant in ("full", "nope")
    do_pe = variant in ("full", "nodve")
    do_tail = variant == "full"
    nc = bacc.Bacc(
        "TRN2",
        target_bir_lowering=False,
        debug=False,
        enable_asserts=False,
        num_devices=num_devices,
    )
    x_d = nc.dram_tensor("x", [BL, N, D], f32, kind="ExternalInput").ap()
    qb_d = nc.dram_tensor("qb", [P, D], f32, kind="ExternalInput").ap()
    hw_d = nc.dram_tensor("hw", [1, D], f32, kind="ExternalInput").ap()
    hb_d = nc.dram_tensor("hb", [1, 1], f32, kind="ExternalInput").ap()
    alpha_d = nc.dram_tensor("alpha", [BL, T, P], f32, kind="ExternalOutput").ap()
    scores_d = nc.dram_tensor("scores", [1, BL], f32, kind="ExternalOutput").ap()

    Alu = mybir.AluOpType
    Act = mybir.ActivationFunctionType

    with tile.TileContext(nc) as tc:
        with (
            tc.tile_pool(name="consts", bufs=1) as consts,
            tc.tile_pool(name="xp", bufs=16) as xp,
            tc.tile_pool(name="lw", bufs=2) as lw,
            tc.tile_pool(name="small", bufs=2) as small,
            tc.tile_pool(name="outp", bufs=2) as outp,
            tc.tile_pool(name="pspool", bufs=2, space="PSUM") as psp,
            tc.tile_pool(name="pssmall", bufs=1, space="PSUM") as pss,
        ):
            qb = consts.tile([P, D], f32)
            nc.scalar.dma_start(qb, qb_d)
            hw = consts.tile([1, D], f32)
            nc.scalar.dma_start(hw, hw_d)
            hb = consts.tile([1, 1], f32)
            nc.scalar.dma_start(hb, hb_d)
            ident = consts.tile([P, P], f32)
            make_identity(nc, ident)
            ones_col = consts.tile([P, 1], f32)
            nc.vector.memset(ones_col, 1.0)
            ones_row = consts.tile([1, P], f32)
            nc.vector.memset(ones_row, 1.0)
            dummy = consts.tile([P, 1], f32)        # throwaway TTR `out`
            scores_sb = consts.tile([1, BL], f32)   # persistent

            def emit_batch(b):
                L = W = pooled_ps = None
                if do_dve:
                    L = lw.tile([P, T], f32, tag="L")
                    W = lw.tile([P, T], f32, tag="W")
                if do_pe:
                    pooled_ps = psp.tile([1, D], f32, tag="pooled")
                for t in range(T):
                    xs = xp.tile([P, D], f32, tag="x")
                    nc.sync.dma_start(xs, x_d[b, t * P:(t + 1) * P, :])
                    if do_dve:
                        nc.vector.scalar_tensor_tensor(
                            out=dummy.broadcast_to((P, D)),
                            in0=xs,
                            scalar=INV_SQRT_D,
                            in1=qb,
                            op0=Alu.mult,
                            op1=Alu.mult,
                            accum_out=L[:, t:t + 1],
                        )
                        nc.scalar.activation(W[:, t:t + 1], L[:, t:t + 1], Act.Exp)
                    if do_pe:
                        wcol = W[:, t:t + 1] if do_dve else ones_col
                        for j in range(2):
                            nc.tensor.matmul(
                                pooled_ps[:, j * 512:(j + 1) * 512],
                                lhsT=wcol,
                                rhs=xs[:, j * 512:(j + 1) * 512],
                                start=(t == 0),
                                stop=(t == T - 1),
                            )

                if not do_tail:
                    if do_pe:
                        pooled_sb = small.tile([1, D], f32, tag="pooled_sb")
                        nc.scalar.copy(pooled_sb, pooled_ps)
                    return

                # ---- per-batch tail ----
                s_col = small.tile([P, 1], f32, tag="s_col")
                nc.vector.tensor_reduce(s_col, W, mybir.AxisListType.X, Alu.add)
                denom_ps = pss.tile([1, 1], f32, tag="denom")
                nc.tensor.matmul(denom_ps, lhsT=ones_col, rhs=s_col, start=True, stop=True)
                denom_sb = small.tile([1, 1], f32, tag="denom_sb")
                nc.scalar.copy(denom_sb, denom_ps)
                recip_sb = small.tile([1, 1], f32, tag="recip_sb")
                nc.vector.reciprocal(recip_sb, denom_sb)
                bcast_ps = pss.tile([P, 1], f32, tag="bcast")
                nc.tensor.matmul(bcast_ps, lhsT=ones_row, rhs=recip_sb, start=True, stop=True)
                recip_col = small.tile([P, 1], f32, tag="recip_col")
                nc.scalar.copy(recip_col, bcast_ps)

                alphaT_ps = psp.tile([T, P], f32, tag="alphaT")
                nc.tensor.transpose(alphaT_ps, W, ident)
                alphaT_sb = outp.tile([T, P], f32, tag="alphaT_sb")
                nc.scalar.activation(
                    alphaT_sb, alphaT_ps, Act.Copy, scale=recip_col[0:T, :]
                )
                nc.scalar.dma_start(alpha_d[b], alphaT_sb)

                pooled_sb = small.tile([1, D], f32, tag="pooled_sb")
                nc.scalar.activation(pooled_sb, pooled_ps, Act.Copy, scale=recip_sb)
                score_col = small.tile([1, 1], f32, tag="score_col")
                nc.vector.scalar_tensor_tensor(
                    out=dummy[0:1, :].broadcast_to((1, D)),
                    in0=pooled_sb,
                    scalar=1.0,
                    in1=hw,
                    op0=Alu.mult,
                    op1=Alu.mult,
                    accum_out=score_col,
                )
                nc.scalar.activation(
                    scores_sb[:, b:b + 1], score_col, Act.Identity, bias=hb, scale=1.0
                )

            def emit_all():
                for b in range(BL):
                    emit_batch(b)
                if do_tail:
                    nc.scalar.dma_start(scores_d, scores_sb)

            if reps == 1:
                emit_all()
            else:
                with tc.For_i(0, reps, 1):
                    emit_all()

    nc.compile()
    return nc


def kernel(x_final, x_full, mask, q, head_w, head_b, trace=False):
    global LAST_RESULTS
    if "nc" not in _CACHE:
        _CACHE["nc"] = build_kernel()
    nc = _CACHE["nc"]

    x_full = np.asarray(x_full, dtype=np.float32)
    q = np.asarray(q, dtype=np.float32).reshape(D)
    head_w = np.asarray(head_w, dtype=np.float32).reshape(D)
    head_b = np.asarray(head_b, dtype=np.float32).reshape(1, 1)

    qb = np.ascontiguousarray(np.broadcast_to(q, (P, D)))
    hw = head_w.reshape(1, D)

    in_maps = []
    for c in range(NCORES):
        in_maps.append(
            {
                "x": x_full[c * BL:(c + 1) * BL],
                "qb": qb,
                "hw": hw,
                "hb": head_b,
            }
        )

    res = run_bass_kernel_spmd(
        nc, in_maps, core_ids=list(range(NCORES)), trace=trace
    )
    LAST_RESULTS = res

    scores = np.concatenate(
        [res.results[c]["scores"].reshape(BL) for c in range(NCORES)]
    )
    alpha = np.concatenate(
        [res.results[c]["alpha"].reshape(BL, N) for c in range(NCORES)], axis=0
    )
    return scores, alpha


# revision 32
# speedup vs baseline: 49321.6578x; 48745.8923x over previous
"""AttentionProbe (B=32, N=4096, D=1024) on 8 Trainium2 NeuronCores.

Sharding: data-parallel over batch B; each core owns 4 batches and streams
its x slice [4, 4096, 1024] f32 (64 MB) through SBUF exactly once.

Per [128, 1024] tile of x (n on partitions, d on free):
  - DVE scalar_tensor_tensor (fused mul+reduce): logits col = sum_d(x/sqrt(D) * q_bcast)
  - ACT exp (no max-subtraction needed: logits ~ N(0, 1/1024), exp in [0.85, 1.15])
  - 2 PE matmuls: pooled[0, 512j:512j+512] += exp(logits).T @ x[:, 512j:512j+512]
    (w stationary [128,1], x moving fp32 [128,512] — accumulated in PSUM
    across the batch's 32 tiles; normalized once at the end)
Per-batch tail: denom via DVE free-reduce + PE ones-matmul partition-sum,
reciprocal on DVE, broadcast back to all partitions via a PE rank-1 matmul,
alpha row = transpose(exp(logits)) on PE, normalized during the ACT
PSUM->SBUF copy, DMA'd out contiguously; scores = pooled . head_w + head_b
via a fused DVE dot + ACT bias. Everything is fp32-exact (measured ~2e-7
rel err vs the fp32 jax reference; ~220 us/core vs ~190 us DMA roofline).
"""

import numpy as np

import concourse.mybir as mybir
import concourse.tile as tile
from concourse import bacc
from concourse.bass_utils import run_bass_kernel_spmd
from concourse.masks import make_identity

B, N, D = 32, 4096, 1024
NCORES = 8
BL = B // NCORES   # batches per core
P = 128
T = N // P         # n-tiles per batch
DJ = D // P        # d-chunks
INV_SQRT_D = 1.0 / 32.0

f32 = mybir.dt.float32

_CACHE = {}
LAST_RESULTS = None


def build_kernel(BL=BL, N=N, D=D, num_devices=NCORES, reps=1, variant="full",
                 pipeline_tail=False, dgroup=1, xbufs=16):
    """variant: 'full' | 'nope' (skip PE work) | 'nodve' (skip STT/exp) |
    'dmaonly' (only the x stream)."""
    T = N // P
    DJ = D // P
    INV_SQRT_D = 1.0 / float(np.sqrt(D))
    do_dve = variant in ("full", "nope")
    do_pe = variant in ("full", "nodve")
    do_tail = variant == "full"
    nc = bacc.Bacc(
        "TRN2",
        target_bir_lowering=False,
        debug=False,
        enable_asserts=False,
        num_devices=num_devices,
    )
    x_d = nc.dram_tensor("x", [BL, N, D], f32, kind="ExternalInput").ap()
    qb_d = nc.dram_tensor("qb", [P, D], f32, kind="ExternalInput").ap()
    hw_d = nc.dram_tensor("hw", [1, D], f32, kind="ExternalInput").ap()
    hb_d = nc.dram_tensor("hb", [1, 1], f32, kind="ExternalInput").ap()
    alpha_d = nc.dram_tensor("alpha", [BL, T, P], f32, kind="ExternalOutput").ap()
    scores_d = nc.dram_tensor("scores", [1, BL], f32, kind="ExternalOutput").ap()

    Alu = mybir.AluOpType
    Act = mybir.ActivationFunctionType

    with tile.TileContext(nc) as tc:
        with (
            tc.tile_pool(name="consts", bufs=1) as consts,
            tc.tile_pool(name="xp", bufs=max(2, xbufs // dgroup)) as xp,
            tc.tile_pool(name="lw", bufs=2) as lw,
            tc.tile_pool(name="small", bufs=2) as small,
            tc.tile_pool(name="outp", bufs=2) as outp,
            tc.tile_pool(name="pspool", bufs=2, space="PSUM") as psp,
            tc.tile_pool(name="pssmall", bufs=1, space="PSUM") as pss,
        ):
            qb = consts.tile([P, D], f32)
            nc.scalar.dma_start(qb, qb_d)
            hw = consts.tile([1, D], f32)
            nc.scalar.dma_start(hw, hw_d)
            hb = consts.tile([1, 1], f32)
            nc.scalar.dma_start(hb, hb_d)
            ident = consts.tile([P, P], f32)
            make_identity(nc, ident)
            ones_col = consts.tile([P, 1], f32)
            nc.vector.memset(ones_col, 1.0)
            ones_row = consts.tile([1, P], f32)
            nc.vector.memset(ones_row, 1.0)
            dummy = consts.tile([P, 1], f32)        # throwaway TTR `out`
            scores_sb = consts.tile([1, BL], f32)   # persistent

            def emit_stream(b):
                L = W = pooled_ps = None
                if do_dve:
                    L = lw.tile([P, T], f32, tag="L")
                    W = lw.tile([P, T], f32, tag="W")
                if do_pe:
                    pooled_ps = psp.tile([1, D], f32, tag="pooled")
                for s_ in range(T // dgroup):
                    xs = xp.tile([P, dgroup * D], f32, tag="x")
                    if dgroup == 1:
                        nc.sync.dma_start(xs, x_d[b, s_ * P:(s_ + 1) * P, :])
                    else:
                        src = x_d[b, s_ * dgroup * P:(s_ + 1) * dgroup * P, :]
                        nc.sync.dma_start(
                            xs.rearrange("p (t d) -> p t d", t=dgroup),
                            src.rearrange("(t p) d -> p t d", p=P),
                        )
                    for ti in range(dgroup):
                        t = s_ * dgroup + ti
                        xt = xs[:, ti * D:(ti + 1) * D]
                        if do_dve:
                            nc.vector.scalar_tensor_tensor(
                                out=dummy.broadcast_to((P, D)),
                                in0=xt,
                                scalar=INV_SQRT_D,
                                in1=qb,
                                op0=Alu.mult,
                                op1=Alu.mult,
                                accum_out=L[:, t:t + 1],
                            )
                            nc.scalar.activation(W[:, t:t + 1], L[:, t:t + 1], Act.Exp)
                        if do_pe:
                            wcol = W[:, t:t + 1] if do_dve else ones_col
                            for j in range(2):
                                nc.tensor.matmul(
                                    pooled_ps[:, j * 512:(j + 1) * 512],
                                    lhsT=wcol,
                                    rhs=xt[:, j * 512:(j + 1) * 512],
                                    start=(t == 0),
                                    stop=(t == T - 1),
                                )

                if not do_tail and do_pe:
                    pooled_sb = small.tile([1, D], f32, tag="pooled_sb")
                    nc.scalar.copy(pooled_sb, pooled_ps)
                return W, pooled_ps

            def emit_tail(b, W, pooled_ps):
                s_col = small.tile([P, 1], f32, tag="s_col")
                nc.vector.tensor_reduce(s_col, W, mybir.AxisListType.X, Alu.add)
                denom_ps = pss.tile([1, 1], f32, tag="denom")
                nc.tensor.matmul(denom_ps, lhsT=ones_col, rhs=s_col, start=True, stop=True)
                denom_sb = small.tile([1, 1], f32, tag="denom_sb")
                nc.scalar.copy(denom_sb, denom_ps)
                recip_sb = small.tile([1, 1], f32, tag="recip_sb")
                nc.vector.reciprocal(recip_sb, denom_sb)
                bcast_ps = pss.tile([P, 1], f32, tag="bcast")
                nc.tensor.matmul(bcast_ps, lhsT=ones_row, rhs=recip_sb, start=True, stop=True)
                recip_col = small.tile([P, 1], f32, tag="recip_col")
                nc.scalar.copy(recip_col, bcast_ps)

                alphaT_ps = psp.tile([T, P], f32, tag="alphaT")
                nc.tensor.transpose(alphaT_ps, W, ident)
                alphaT_sb = outp.tile([T, P], f32, tag="alphaT_sb")
                nc.scalar.activation(
                    alphaT_sb, alphaT_ps, Act.Copy, scale=recip_col[0:T, :]
                )
                nc.scalar.dma_start(alpha_d[b], alphaT_sb)

                pooled_sb = small.tile([1, D], f32, tag="pooled_sb")
                nc.scalar.activation(pooled_sb, pooled_ps, Act.Copy, scale=recip_sb)
                score_col = small.tile([1, 1], f32, tag="score_col")
                gdummy = small.tile([1, 1], f32, tag="gdummy")
                nc.vector.scalar_tensor_tensor(
                    out=gdummy.broadcast_to((1, D)),
                    in0=pooled_sb,
                    scalar=1.0,
                    in1=hw,
                    op0=Alu.mult,
                    op1=Alu.mult,
                    accum_out=score_col,
                )
                nc.scalar.activation(
                    scores_sb[:, b:b + 1], score_col, Act.Identity, bias=hb, scale=1.0
                )

            def emit_all():
                # pipeline_tail: batch b's tail is emitted after batch b+1's
                # streaming ops so its cross-engine chain never stalls the
                # per-tile DVE/ACT/PE streams.
                pending = None
                for b in range(BL):
                    state = emit_stream(b)
                    if do_tail and not pipeline_tail:
                        emit_tail(b, *state)
                        continue
                    if do_tail and pending is not None:
                        emit_tail(pending[0], *pending[1])
                    pending = (b, state)
                if do_tail and pipeline_tail and pending is not None:
                    emit_tail(pending[0], *pending[1])
                if do_tail:
                    nc.scalar.dma_start(scores_d, scores_sb)

            if reps == 1:
                emit_all()
            else:
                with tc.For_i(0, reps, 1):
                    emit_all()

    nc.compile()
    return nc


def kernel(x_final, x_full, mask, q, head_w, head_b, trace=False):
    global LAST_RESULTS
    if "nc" not in _CACHE:
        _CACHE["nc"] = build_kernel()
    nc = _CACHE["nc"]

    x_full = np.asarray(x_full, dtype=np.float32)
    q = np.asarray(q, dtype=np.float32).reshape(D)
    head_w = np.asarray(head_w, dtype=np.float32).reshape(D)
    head_b = np.asarray(head_b, dtype=np.float32).reshape(1, 1)

    qb = np.ascontiguousarray(np.broadcast_to(q, (P, D)))
    hw = head_w.reshape(1, D)

    in_maps = []
    for c in range(NCORES):
        in_maps.append(
            {
                "x": x_full[c * BL:(c + 1) * BL],
                "qb": qb,
                "hw": hw,
                "hb": head_b,
            }
        )

    res = run_bass_kernel_spmd(
        nc, in_maps, core_ids=list(range(NCORES)), trace=trace
    )
    LAST_RESULTS = res

    scores = np.concatenate(
        [res.results[c]["scores"].reshape(BL) for c in range(NCORES)]
    )
    alpha = np.concatenate(
        [res.results[c]["alpha"].reshape(BL, N) for c in range(NCORES)], axis=0
    )
    return scores, alpha
